# revision 19
# baseline (speedup 1.0000x reference)
"""Trainium2 Bass kernel: multi-head attention (B=4, S=2048, D=1024, H=16, HD=64).

Sharding: 8 cores = 4 batches x 2 head-groups (8 heads each).
Each core computes, for its (batch b, head-group g):
    qT/kT (RoPE'd, RMS-normed, scale-folded) via projections with
    host-pre-transposed inputs/weights, v in natural layout (bf16),
    causal flash-style attention (no max subtraction; fp32 range is
    ample), and a partial output projection with the group's Wo rows.
Host sums the two partial outputs per batch.

v2 schedule: resident weights, K=64 row-tiled score pairs, h2-batched
exp into 2-bank PSUM tiles, affine_select triangle masks, ln/exp rms
(single ACT table set), and proj(j+1) emission inside chunk j's
softmax-denominator window with outproj/proj_v as rms-stall fillers.
"""

import math
import os
from contextlib import ExitStack

import numpy as np

import concourse.bacc as bacc
import concourse.bass as bass
import concourse.mybir as mybir
import concourse.tile as tile
from concourse.bass_utils import run_bass_kernel_spmd

try:
    from neuron_dtypes._impl.fp32r import cast_fp32_to_fp32r as _c32r
except Exception:  # pragma: no cover
    _c32r = None


def _round_fp32r(a):
    """Round fp32 array to the fp32r encoding the PE consumes (TF32-like)."""
    a = np.ascontiguousarray(a, np.float32)
    if _c32r is None:
        u = a.view(np.uint32)
        low = u & 0xFFF
        u = (u & ~np.uint32(0xFFF)) + np.where(
            (low > 0x800) | ((low == 0x800) & ((u >> 12) & 1).astype(bool)),
            np.uint32(0x1000), np.uint32(0))
        return u.view(np.float32)
    flat = a.reshape(-1).view(np.uint32)
    out = _c32r(flat.size, flat)
    return np.asarray(out, np.uint32).reshape(a.shape).view(np.float32)

B, D, H, HD = 4, 1024, 16, 64
S_FULL = 2048
HALF = 32          # rope pair offset within a head
GH = 8             # heads per core (head-group)
GO = GH * HD       # 512 projection dims per group
EPS = 1e-6
LOG2_E = 1.442695041
N_CORES = 8
P = 128            # partitions
CH = 512           # s-chunk width (matmul free dim)
KT = D // P        # 8 contraction tiles
NT = GO // P       # 4 partition tiles of the group's 512 dims
F32 = mybir.dt.float32
F32R = mybir.dt.float32r
BF16 = mybir.dt.bfloat16
MULT = mybir.AluOpType.mult

LAST_RESULTS = None  # BassKernelResults of the most recent run (for profiling)


def build_bass(s=S_FULL):
    nch = s // CH          # s-chunks

    nc = bacc.Bacc("TRN2", target_bir_lowering=False, debug=False)

    def _mm(out, lhsT, rhs, start=True, stop=True):
        nc.tensor.matmul(
            out, lhsT.bitcast(F32R), rhs.bitcast(F32R), start=start, stop=stop
        )

    xT = nc.dram_tensor("xT", [D, s], F32R, kind="ExternalInput").ap()
    wqT = nc.dram_tensor("wqT", [D, GO], F32R, kind="ExternalInput").ap()
    wkT = nc.dram_tensor("wkT", [D, GO], F32R, kind="ExternalInput").ap()
    wvT = nc.dram_tensor("wvT", [D, GO], F32R, kind="ExternalInput").ap()
    woT = nc.dram_tensor("woT", [GO, D], F32R, kind="ExternalInput").ap()
    cosT = nc.dram_tensor("cosT", [P, s], F32, kind="ExternalInput").ap()
    sinT = nc.dram_tensor("sinT", [P, s], F32, kind="ExternalInput").ap()
    fnat = nc.dram_tensor("fnat", [P, 1], F32, kind="ExternalInput").ap()
    msq = nc.dram_tensor("msq", [2, P, GH], F32R, kind="ExternalInput").ap()
    mR = nc.dram_tensor("mR", [NT, GH, P], F32R, kind="ExternalInput").ap()
    mP = nc.dram_tensor("mP", [NT, 2, P, P], F32R, kind="ExternalInput").ap()
    zerod = nc.dram_tensor("zerod", [HD, CH], F32R, kind="ExternalInput").ap()
    out = nc.dram_tensor("out", [s, D], F32, kind="ExternalOutput").ap()

    with nc.allow_low_precision(reason="fp32r/bf16 rounding is intentional"), \
            tile.TileContext(nc) as tc, ExitStack() as ctx:
        consts = ctx.enter_context(tc.tile_pool(name="consts", bufs=1))
        wqpool = ctx.enter_context(tc.tile_pool(name="wqpool", bufs=1))
        wkpool = ctx.enter_context(tc.tile_pool(name="wkpool", bufs=1))
        wvpool = ctx.enter_context(tc.tile_pool(name="wvpool", bufs=1))
        wopool = ctx.enter_context(tc.tile_pool(name="wopool", bufs=1))
        xpool = ctx.enter_context(tc.tile_pool(name="xpool", bufs=8))
        cspool = ctx.enter_context(tc.tile_pool(name="cspool", bufs=1))
        qrpool = ctx.enter_context(tc.tile_pool(name="qrpool", bufs=4))
        sqpool = ctx.enter_context(tc.tile_pool(name="sqpool", bufs=4))
        rqpool = ctx.enter_context(tc.tile_pool(name="rqpool", bufs=2))
        bqpool = ctx.enter_context(tc.tile_pool(name="bqpool", bufs=1))
        qnpool = ctx.enter_context(tc.tile_pool(name="qnpool", bufs=8))
        knpool = ctx.enter_context(tc.tile_pool(name="knpool", bufs=4 * nch))
        vpool = ctx.enter_context(tc.tile_pool(name="vpool", bufs=4 * nch))
        ppool = ctx.enter_context(tc.tile_pool(name="ppool", bufs=3))
        rspool = ctx.enter_context(tc.tile_pool(name="rspool", bufs=1))
        obpool = ctx.enter_context(tc.tile_pool(name="obpool", bufs=1))
        cxpool = ctx.enter_context(tc.tile_pool(name="cxpool", bufs=4))
        psum = ctx.enter_context(tc.tile_pool(name="psum", bufs=2, space="PSUM"))

        # --- tiny constants ---
        zb = consts.tile([P, 1], F32, tag="zb", name="zb")
        nc.vector.memset(zb, 0.0)
        epsb = consts.tile([P, 1], F32, tag="epsb", name="epsb")
        nc.vector.memset(epsb, EPS)
        f_sb = consts.tile([P, 1], F32, tag="f_sb", name="f_sb")
        nc.sync.dma_start(out=f_sb, in_=fnat)

        msq_sb, mR_sb, mP_sb = [], [], []

        def load_small_consts():
            for hg in range(2):
                t_ = consts.tile([P, GH], F32, tag=f"msq{hg}", name=f"msq{hg}")
                nc.sync.dma_start(out=t_.bitcast(F32R), in_=msq[hg])
                msq_sb.append(t_)
            for t in range(NT):
                t_ = consts.tile([GH, P], F32, tag=f"mR{t}", name=f"mR{t}")
                nc.sync.dma_start(out=t_.bitcast(F32R), in_=mR[t])
                mR_sb.append(t_)
            for t in range(NT):
                row = []
                for sr in range(2):
                    t_ = consts.tile([P, P], F32, tag=f"mP{t}_{sr}",
                                     name=f"mP{t}_{sr}")
                    nc.sync.dma_start(out=t_.bitcast(F32R), in_=mP[t, sr])
                    row.append(t_)
                mP_sb.append(row)

        def dummy_mms(n):
            # tiny dependency-free matmuls that keep the PE activity
            # monitor busy through short stalls (HAM re-throttles on
            # sub-us idles and then halves the PE clock for 3.4us+)
            for _ in range(n):
                ds = psum.tile([32, 64], F32, tag="ps", name="ds")
                _mm(ds, mP_sb[0][0][:, 0:32], mP_sb[0][1][:, 0:64])

        # --- resident weights / streamed per-chunk tiles ---
        wq_sb, wk_sb, wv_sb, wo_sb = [], [], [], []
        x_tiles = {}     # j -> list of 8 [P, CH] tiles
        cs_tiles = {}    # j -> (cos, sin)
        kn_t = {}        # (t, jc) -> [P, CH]: c-dims (2 heads) x k-positions
        qn_t = {}        # (t, j) -> [P, CH]
        v_t = {}         # s-tile -> [P, GH, HD+1] bf16 (ones col appended)

        def load_x(j):
            xt = []
            for k in range(KT):
                t_ = xpool.tile([P, CH], F32, tag="xt", name="xt")
                nc.sync.dma_start(
                    out=t_.bitcast(F32R),
                    in_=xT[k * P:(k + 1) * P, j * CH:(j + 1) * CH])
                xt.append(t_)
            x_tiles[j] = xt
            cos_sb = cspool.tile([P, CH], F32, tag="cos", name="cos")
            nc.sync.dma_start(out=cos_sb, in_=cosT[:, j * CH:(j + 1) * CH])
            sin_sb = cspool.tile([P, CH], F32, tag="sin", name="sin")
            nc.sync.dma_start(out=sin_sb, in_=sinT[:, j * CH:(j + 1) * CH])
            cs_tiles[j] = (cos_sb, sin_sb)

        def projA_steps(j, w_sb):
            """Phase A of a q/k projection as 4 filler thunks (one per psq
            slice: 8 matmuls, plus rope + squares after odd slices).  Only
            touches the 'ps' PSUM ring so the thunks are safe to emit inside
            an attention k-loop.  Returns (thunks, state)."""
            st = {"qr": [None] * NT, "sq": [None] * NT, "pair": [None, None]}

            def step(m):
                xt = x_tiles[j]
                cos_sb, sin_sb = cs_tiles[j]
                ps = psum.tile([P, CH], F32, tag="ps", name="ps")
                for k in range(KT):
                    _mm(ps, w_sb[k][:, m * P:(m + 1) * P], xt[k],
                        start=(k == 0), stop=(k == KT - 1))
                st["pair"][m % 2] = ps
                if m % 2 == 1:
                    a, b = st["pair"]
                    t1 = qrpool.tile([P, CH], F32, tag="qr", name="qr")
                    nc.vector.tensor_tensor(t1.bitcast(F32R), a, cos_sb, MULT)
                    t4 = qrpool.tile([P, CH], F32, tag="rtmp", name="rtmp",
                                     bufs=1)
                    nc.vector.tensor_tensor(t4, a, sin_sb, MULT)
                    t2 = sqpool.tile([P, CH], F32, tag="rtmp2", name="rtmp2",
                                     bufs=1)
                    nc.vector.tensor_tensor(t2, b, sin_sb, MULT)
                    t3 = qrpool.tile([P, CH], F32, tag="qr", name="qr")
                    nc.vector.tensor_tensor(t3.bitcast(F32R), b, cos_sb, MULT)
                    nc.vector.tensor_sub(t1.bitcast(F32R), t1, t2)
                    nc.vector.tensor_add(t3.bitcast(F32R), t3, t4)
                    st["qr"][m - 1], st["qr"][m] = t1, t3
                    for mq in (m - 1, m):
                        s_ = sqpool.tile([P, CH], F32, tag="sq", name="sq")
                        nc.vector.tensor_mul(s_.bitcast(F32R), st["qr"][mq],
                                             st["qr"][mq])
                        st["sq"][mq] = s_

            return [lambda m=m: step(m) for m in range(NT)], st

        def phaseB(j, st, is_k):
            """RMS (ln->exp(-0.5*)) + repermute + scale for one path.
            Uses the 'pv' PSUM ring: only emit at t-loop boundaries."""
            qr, sq = st["qr"], st["sq"]
            pss = psum.tile([GH, CH], F32, tag="pv", name="pss")
            for m in range(NT):
                _mm(pss, msq_sb[m // 2], sq[m],
                    start=(m == 0), stop=(m == NT - 1))
            lnv = rqpool.tile([GH, CH], F32, tag="lnv", name="lnv", bufs=1)
            nc.scalar.activation(lnv, pss, mybir.ActivationFunctionType.Ln,
                                 bias=epsb[0:GH], scale=1.0 / HD)
            rqcR = rqpool.tile([GH, CH], F32, tag="rqR", name="rqR", bufs=2)
            nc.scalar.activation(rqcR.bitcast(F32R), lnv,
                                 mybir.ActivationFunctionType.Exp,
                                 bias=zb[0:GH], scale=-0.5)
            for t in range(NT):
                psb = psum.tile([P, CH], F32, tag="pv", name="psb")
                _mm(psb, mR_sb[t], rqcR)
                rqs = bqpool.tile([P, CH], F32, tag="bq", name="bq")
                nc.vector.tensor_copy(rqs, psb)
                psr = psum.tile([P, CH], F32, tag="pv", name="psr")
                _mm(psr, mP_sb[t][0], qr[2 * (t // 2)], start=True, stop=False)
                _mm(psr, mP_sb[t][1], qr[2 * (t // 2) + 1],
                    start=False, stop=True)
                if is_k:
                    dst = knpool.tile([P, CH], F32, tag="kn", name="kn")
                    nc.vector.scalar_tensor_tensor(
                        dst.bitcast(F32R), in0=psr, scalar=f_sb, in1=rqs,
                        op0=MULT, op1=MULT)
                    kn_t[(t, j)] = dst
                else:
                    # per-head zero-padded tiles: head data at its kn
                    # partition range, zeros elsewhere (K=128 score matmuls
                    # at full rate; K=64 row-tiled pairs measured 1.5x
                    # slower with fp32r)
                    for h2 in range(2):
                        po = HD * h2
                        qz = qnpool.tile([P, CH], F32, tag="qn", name="qn")
                        nc.sync.dma_start(
                            out=qz[HD - po:2 * HD - po, :].bitcast(F32R),
                            in_=zerod)
                        nc.vector.tensor_tensor(
                            qz[po:po + HD, :].bitcast(F32R),
                            psr[po:po + HD, :], rqs[po:po + HD, :], MULT)
                        qn_t[(2 * t + h2, j)] = qz

        def projv_steps(j):
            def step(si):
                xt = x_tiles[j]
                ps = psum.tile([P, CH], F32, tag="ps", name="ps")
                for k in range(KT):
                    _mm(ps, xt[k][:, si * P:(si + 1) * P], wv_sb[k],
                        start=(k == 0), stop=(k == KT - 1))
                vt = vpool.tile([P, GH, HD + 1], BF16, tag="vt", name="vt")
                nc.gpsimd.memset(vt[:, :, HD:HD + 1], 1.0)
                nc.scalar.copy(vt[:, :, 0:HD],
                               ps.rearrange("p (h d) -> p h d", h=GH))
                v_t[j * NT + si] = vt

            return [lambda si=si: step(si) for si in range(NT)]

        def attention(j, fillers=(), hooks=()):
            """Causal attention for chunk j's queries -> (ctx tiles, rr).

            fillers: sorted [(slot, thunk)] popped as progress (t + frac)
            passes each slot — next chunk's projection phase-A work that
            keeps the PE dense while the k-loop is exp-paced.  hooks: {t:
            thunk} run at t-loop boundaries (phase-B work on the 'pv' ring,
            which is only WAR-safe between t iterations)."""
            kmax = 4 * j + 3
            fq = list(fillers)
            hooks = dict(hooks)
            rr = rqpool.tile([GH, CH], F32, tag="rr", name="rr", bufs=1)
            ctx_t = [cxpool.tile([P, CH], F32, tag="cx", name="cx")
                     for _ in range(NT)]
            for t in range(NT):
                while fq and fq[0][0] <= t:
                    fq.pop(0)[1]()
                if t in hooks:
                    hooks[t]()
                    dummy_mms(14)
                pvs = [psum.tile([HD + 1, CH], F32, tag="pv", name="pv")
                       for _ in range(2)]
                LAG = 2
                pending = {}

                def emit_pv(kk):
                    c0k, p3k = pending.pop(kk)
                    for h2 in range(2):
                        nc.tensor.matmul(
                            pvs[h2][:, c0k:], v_t[kk][:, 2 * t + h2, :],
                            p3k[:, h2, c0k:],
                            start=(kk == 0), stop=(kk == kmax))

                for k in range(kmax + 1):
                    c0 = max(0, 128 * k - CH * j)
                    kt = kn_t[(t, k // 4)]
                    kwin = slice((k % 4) * P, (k % 4) * P + P)
                    ssp = psum.tile([P, 2, CH], F32, tag="ssp", name="ssp")
                    for h2 in range(2):
                        _mm(ssp[:, h2, c0:], kt[:, kwin],
                            qn_t[(2 * t + h2, j)][:, c0:])
                    p3 = ppool.tile([P, 2, CH], BF16, tag="pp", name="pp")
                    nc.scalar.activation(p3[:, :, c0:], ssp[:, :, c0:],
                                         mybir.ActivationFunctionType.Exp,
                                         bias=zb, scale=1.0)
                    if k >= 4 * j:
                        # in-block causal triangle: zero q < k entries
                        nc.gpsimd.affine_select(
                            out=p3[:, :, c0:c0 + P], in_=p3[:, :, c0:c0 + P],
                            pattern=[[0, 2], [1, P]],
                            compare_op=mybir.AluOpType.is_ge,
                            fill=0.0, base=0, channel_multiplier=-1)
                    pending[k] = (c0, p3)
                    if k >= LAG:
                        emit_pv(k - LAG)
                    prog = t + (k + 1.0) / (kmax + 1)
                    popped = False
                    while fq and fq[0][0] <= prog:
                        fq.pop(0)[1]()
                        popped = True
                    if not fq and not popped:
                        dummy_mms(3)
                for kk in range(max(0, kmax + 1 - LAG), kmax + 1):
                    emit_pv(kk)
                for h2 in range(2):
                    hl, po = 2 * t + h2, HD * h2
                    nc.vector.tensor_copy(
                        ctx_t[t][po:po + HD, :].bitcast(F32R),
                        pvs[h2][0:HD, :])
                    rs = rspool.tile([1, CH], F32, tag="rs", name="rs")
                    nc.vector.tensor_copy(rs, pvs[h2][HD:HD + 1, :])
                    nc.sync.dma_start(out=rr[hl:hl + 1, :], in_=rs)
            for _, thunk in fq:
                thunk()
            return ctx_t, rr

        def denom_recip(rr):
            rscr = rqpool.tile([GH, CH], F32, tag="lnv", name="rscr", bufs=1)
            rrc = rqpool.tile([GH, CH], F32, tag="rrc", name="rrc", bufs=1)
            nc.vector.reciprocal_approx_accurate(out=rrc, in_=rr, scratch=rscr)
            rrR = rqpool.tile([GH, CH], F32, tag="rqR", name="rrR", bufs=2)
            nc.vector.tensor_copy(rrR.bitcast(F32R), rrc)
            return rrR

        def denom_apply(ctx_t, rrR):
            for t in range(NT):
                psn = psum.tile([P, CH], F32, tag="ps", name="psn")
                _mm(psn, mR_sb[t], rrR)
                nc.vector.tensor_tensor(ctx_t[t].bitcast(F32R), psn,
                                        ctx_t[t], MULT)

        def outproj(j, ctx_t):
            for si in range(NT):
                for oc in range(2):
                    pso = psum.tile([P, CH], F32, tag="ps", name="pso")
                    for ct in range(NT):
                        _mm(pso, ctx_t[ct][:, si * P:(si + 1) * P],
                            wo_sb[ct][:, oc * CH:(oc + 1) * CH],
                            start=(ct == 0), stop=(ct == NT - 1))
                    ob = obpool.tile([P, CH], F32, tag="ob", name="ob")
                    nc.scalar.copy(ob, pso)
                    nc.sync.dma_start(
                        out=out[(j * NT + si) * P:(j * NT + si + 1) * P,
                                oc * CH:(oc + 1) * CH],
                        in_=ob)

        # ---- bootstrap: chunk 0 ----
        load_small_consts()
        # wq/x next so chunk 0's q-projection starts ASAP
        x0 = []
        for k in range(KT):
            t_ = wqpool.tile([P, GO], F32, tag=f"wq{k}", name=f"wq{k}")
            nc.sync.dma_start(out=t_.bitcast(F32R),
                              in_=wqT[k * P:(k + 1) * P, :])
            wq_sb.append(t_)
            t_ = xpool.tile([P, CH], F32, tag="xt", name="xt")
            nc.sync.dma_start(out=t_.bitcast(F32R),
                              in_=xT[k * P:(k + 1) * P, 0:CH])
            x0.append(t_)
        x_tiles[0] = x0
        cos_sb = cspool.tile([P, CH], F32, tag="cos", name="cos")
        nc.sync.dma_start(out=cos_sb, in_=cosT[:, 0:CH])
        sin_sb = cspool.tile([P, CH], F32, tag="sin", name="sin")
        nc.sync.dma_start(out=sin_sb, in_=sinT[:, 0:CH])
        cs_tiles[0] = (cos_sb, sin_sb)
        for k in range(KT):
            t_ = wkpool.tile([P, GO], F32, tag=f"wk{k}", name=f"wk{k}")
            nc.sync.dma_start(out=t_.bitcast(F32R),
                              in_=wkT[k * P:(k + 1) * P, :])
            wk_sb.append(t_)
        for k in range(KT):
            t_ = wvpool.tile([P, GO], F32, tag=f"wv{k}", name=f"wv{k}")
            nc.sync.dma_start(out=t_.bitcast(F32R),
                              in_=wvT[k * P:(k + 1) * P, :])
            wv_sb.append(t_)
        qA, qst = projA_steps(0, wq_sb)
        for th in qA:
            th()
        phaseB(0, qst, is_k=False)
        kA, kst = projA_steps(0, wk_sb)
        for th in kA:
            th()
        phaseB(0, kst, is_k=True)
        for th in projv_steps(0):
            th()
        for ct in range(NT):
            t_ = wopool.tile([P, D], F32, tag=f"wo{ct}", name=f"wo{ct}")
            nc.sync.dma_start(out=t_.bitcast(F32R),
                              in_=woT[ct * P:(ct + 1) * P, :])
            wo_sb.append(t_)

        # ---- main loop ----
        for j in range(nch):
            fillers, hooks, qst = [], {}, None
            if j + 1 < nch:
                load_x(j + 1)          # prefetch next chunk's x + cos/sin
                kA, kst = projA_steps(j + 1, wk_sb)
                qA, qst = projA_steps(j + 1, wq_sb)
                vS = projv_steps(j + 1)
                fillers = sorted(
                    [(0.0 + (i + 1) * 0.5, th) for i, th in enumerate(kA)]
                    + [(1.25 + (i + 1) * 0.5, th) for i, th in enumerate(qA)]
                    + [(2.25 + (i + 1) * 0.5, th) for i, th in enumerate(vS)],
                    key=lambda x: x[0])
                jn = j + 1
                hooks = {2: (lambda jn=jn, kst=kst:
                             phaseB(jn, kst, is_k=True))}
            ctx_t, rr = attention(j, fillers, hooks)
            dummy_mms(14)
            rrR = denom_recip(rr)
            if qst is not None:
                phaseB(j + 1, qst, is_k=False)  # fills the recip window
            denom_apply(ctx_t, rrR)
            outproj(j, ctx_t)
            dummy_mms(10)

    nc.compile()
    return nc


# ---------------------------------------------------------------------------
# Host-side preparation
# ---------------------------------------------------------------------------

def _softplus(x):
    return np.logaddexp(0.0, x)


def _host_tables(s, q_ln_scale, k_ln_scale, per_dim_scale):
    pos = np.arange(s, dtype=np.float64)
    i = np.arange(HALF, dtype=np.float64)
    timescale = 10000.0 ** (2.0 * i / HD)
    ang = pos[None, :] / timescale[:, None]          # [32, s]
    cosT = np.tile(np.cos(ang), (4, 1)).astype(np.float32)  # [128, s]
    sinT = np.tile(np.sin(ang), (4, 1)).astype(np.float32)

    hd = np.arange(P) % HD
    f = (q_ln_scale[hd] * k_ln_scale[hd]
         * (LOG2_E / math.sqrt(HD))
         * _softplus(per_dim_scale[hd].astype(np.float64))).astype(np.float32)
    fnat = f.reshape(P, 1)

    # o2' layout: slice m of the 512 group dims holds half b=m%2 of heads
    # 4*(m//2)..4*(m//2)+3; within a slice, r = 32*hl_local + i.
    msq = np.zeros((2, P, GH), np.float32)
    for hg in range(2):
        for r in range(P):
            msq[hg, r, 4 * hg + r // HALF] = 1.0

    mR = np.zeros((NT, GH, P), np.float32)
    for t in range(NT):
        for m in range(P):
            mR[t, (128 * t + m) // HD, m] = 1.0

    mP = np.zeros((NT, 2, P, P), np.float32)
    for t in range(NT):
        for p in range(P):
            n = 128 * t + p
            hl, d = n // HD, n % HD
            b, i_ = d // HALF, d % HALF
            mP[t, b, 32 * (hl % 4) + i_, p] = 1.0

    return cosT, sinT, fnat, msq, mR, mP


def _oprime_perm():
    """o2'[j] -> natural local dim, for one head group (512 dims)."""
    perm = np.zeros(GO, np.int64)
    for j in range(GO):
        sl, r = j // 128, j % 128
        b, hg = sl % 2, sl // 2
        hl, i = 4 * hg + r // HALF, r % HALF
        perm[j] = HD * hl + HALF * b + i
    return perm


def _numpy_reference(inputs_q, Wq, Wk, Wv, Wo, q_ln_scale, k_ln_scale,
                     per_dim_scale, patch_mask):
    """Exact numpy replica of the reference (fallback for patch_mask != 0)."""
    b, s, d = inputs_q.shape
    x = inputs_q.astype(np.float32)
    q = (x @ Wq.T).reshape(b, s, H, HD)
    k = (x @ Wk.T).reshape(b, s, H, HD)
    v = (x @ Wv.T).reshape(b, s, H, HD)
    num_masked = patch_mask.astype(np.int64).sum(-1)
    position = np.arange(s)[None, :] - num_masked[:, None]

    def rope(t):
        frac = 2.0 * np.arange(HALF) / HD
        ts = 10000.0 ** frac
        ang = position[:, :, None, None].astype(np.float32) / ts[None, None, None, :]
        sin, cos = np.sin(ang), np.cos(ang)
        f, sec = t[..., :HALF], t[..., HALF:]
        return np.concatenate([f * cos - sec * sin, sec * cos + f * sin], -1)

    def rms(t, scale):
        var = np.mean(np.square(t), -1, keepdims=True)
        return t / np.sqrt(var + EPS) * scale

    q = rms(rope(q), q_ln_scale)
    k = rms(rope(k), k_ln_scale)
    q = q * (LOG2_E / math.sqrt(HD) * _softplus(per_dim_scale)).astype(np.float32)
    scores = np.einsum("bqhd,bkhd->bhqk", q, k)
    qi = np.arange(s)[None, None, :, None]
    ki = np.arange(s)[None, None, None, :]
    mask = (qi >= ki) & (ki >= num_masked[:, None, None, None])
    neg = -np.finfo(np.float32).max / 2
    scores = np.where(mask, scores, neg)
    scores = scores - scores.max(-1, keepdims=True)
    e = np.exp(scores)
    attn = e / e.sum(-1, keepdims=True)
    o = np.einsum("bhqk,bkhd->bqhd", attn, v).reshape(b, s, d)
    return (o @ Wo.T).astype(np.float32)


_NC_CACHE = {}


def _get_nc(s):
    if s not in _NC_CACHE:
        _NC_CACHE[s] = build_bass(s)
    return _NC_CACHE[s]


def make_in_maps(inputs_q, Wq, Wk, Wv, Wo, q_ln_scale, k_ln_scale,
                 per_dim_scale, s):
    cosT, sinT, fnat, msq, mR, mP = _host_tables(
        s, np.asarray(q_ln_scale, np.float32),
        np.asarray(k_ln_scale, np.float32),
        np.asarray(per_dim_scale, np.float32))
    perm = _oprime_perm()

    xT = [_round_fp32r(np.asarray(inputs_q[b], np.float32).T)
          for b in range(inputs_q.shape[0])]
    wq_g, wk_g, wv_g, wo_g = [], [], [], []
    for g in range(2):
        rows = g * GO + perm
        wq_g.append(_round_fp32r(np.asarray(Wq, np.float32)[rows, :].T))
        wk_g.append(_round_fp32r(np.asarray(Wk, np.float32)[rows, :].T))
        sl = slice(g * GO, (g + 1) * GO)
        wv_g.append(_round_fp32r(np.asarray(Wv, np.float32)[sl, :].T))
        wo_g.append(_round_fp32r(np.asarray(Wo, np.float32)[:, sl].T))

    in_maps = []
    for c in range(N_CORES):
        b, g = (c // 2) % len(xT), c % 2
        in_maps.append({
            "xT": xT[b], "wqT": wq_g[g], "wkT": wk_g[g], "wvT": wv_g[g],
            "woT": wo_g[g], "cosT": cosT, "sinT": sinT, "fnat": fnat,
            "msq": msq, "mR": mR, "mP": mP,
            "zerod": np.zeros((HD, CH), np.float32),
        })
    return in_maps


def kernel(inputs_q, Wq, Wk, Wv, Wo, q_ln_scale, k_ln_scale,
           per_dim_scale, patch_mask):
    global LAST_RESULTS
    inputs_q = np.asarray(inputs_q, np.float32)
    patch_mask = np.asarray(patch_mask)
    if patch_mask.astype(np.int64).sum() != 0:
        return _numpy_reference(
            inputs_q, np.asarray(Wq, np.float32), np.asarray(Wk, np.float32),
            np.asarray(Wv, np.float32), np.asarray(Wo, np.float32),
            np.asarray(q_ln_scale, np.float32),
            np.asarray(k_ln_scale, np.float32),
            np.asarray(per_dim_scale, np.float32), patch_mask)

    s = inputs_q.shape[1]
    in_maps = make_in_maps(inputs_q, Wq, Wk, Wv, Wo, q_ln_scale, k_ln_scale,
                           per_dim_scale, s)
    nc = _get_nc(s)
    res = run_bass_kernel_spmd(
        nc, in_maps, core_ids=list(range(N_CORES)),
        trace=bool(os.environ.get("KERNEL_TRACE")),
        tmpdir=os.environ.get("KERNEL_TMPDIR") or None,
    )
    LAST_RESULTS = res
    outs = [r["out"] for r in res.results]
    full = np.empty((inputs_q.shape[0], s, D), np.float32)
    for b in range(inputs_q.shape[0]):
        full[b] = outs[2 * b] + outs[2 * b + 1]
    return full


# revision 20
# speedup vs baseline: 1.0174x; 1.0174x over previous
"""Trainium2 Bass kernel: multi-head attention (B=4, S=2048, D=1024, H=16, HD=64).

Sharding: 8 cores = 4 batches x 2 head-groups (8 heads each).
Each core computes, for its (batch b, head-group g):
    qT/kT (RoPE'd, RMS-normed, scale-folded) via projections with
    host-pre-transposed inputs/weights, v in natural layout (bf16),
    causal flash-style attention (no max subtraction; fp32 range is
    ample), and a partial output projection with the group's Wo rows.
Host sums the two partial outputs per batch.

v2 schedule: resident weights, K=64 row-tiled score pairs, h2-batched
exp into 2-bank PSUM tiles, affine_select triangle masks, ln/exp rms
(single ACT table set), and proj(j+1) emission inside chunk j's
softmax-denominator window with outproj/proj_v as rms-stall fillers.
"""

import math
import os
from contextlib import ExitStack

import numpy as np

import concourse.bacc as bacc
import concourse.bass as bass
import concourse.mybir as mybir
import concourse.tile as tile
from concourse.bass_utils import run_bass_kernel_spmd

try:
    from neuron_dtypes._impl.fp32r import cast_fp32_to_fp32r as _c32r
except Exception:  # pragma: no cover
    _c32r = None


def _round_fp32r(a):
    """Round fp32 array to the fp32r encoding the PE consumes (TF32-like)."""
    a = np.ascontiguousarray(a, np.float32)
    if _c32r is None:
        u = a.view(np.uint32)
        low = u & 0xFFF
        u = (u & ~np.uint32(0xFFF)) + np.where(
            (low > 0x800) | ((low == 0x800) & ((u >> 12) & 1).astype(bool)),
            np.uint32(0x1000), np.uint32(0))
        return u.view(np.float32)
    flat = a.reshape(-1).view(np.uint32)
    out = _c32r(flat.size, flat)
    return np.asarray(out, np.uint32).reshape(a.shape).view(np.float32)

B, D, H, HD = 4, 1024, 16, 64
S_FULL = 2048
HALF = 32          # rope pair offset within a head
GH = 8             # heads per core (head-group)
GO = GH * HD       # 512 projection dims per group
EPS = 1e-6
LOG2_E = 1.442695041
N_CORES = 8
P = 128            # partitions
CH = 512           # s-chunk width (matmul free dim)
KT = D // P        # 8 contraction tiles
NT = GO // P       # 4 partition tiles of the group's 512 dims
F32 = mybir.dt.float32
F32R = mybir.dt.float32r
BF16 = mybir.dt.bfloat16
MULT = mybir.AluOpType.mult

LAST_RESULTS = None  # BassKernelResults of the most recent run (for profiling)


def build_bass(s=S_FULL):
    nch = s // CH          # s-chunks

    nc = bacc.Bacc("TRN2", target_bir_lowering=False, debug=False)

    def _mm(out, lhsT, rhs, start=True, stop=True):
        nc.tensor.matmul(
            out, lhsT.bitcast(F32R), rhs.bitcast(F32R), start=start, stop=stop
        )

    xT = nc.dram_tensor("xT", [D, s], F32R, kind="ExternalInput").ap()
    wqT = nc.dram_tensor("wqT", [D, GO], F32R, kind="ExternalInput").ap()
    wkT = nc.dram_tensor("wkT", [D, GO], F32R, kind="ExternalInput").ap()
    wvT = nc.dram_tensor("wvT", [D, GO], F32R, kind="ExternalInput").ap()
    woT = nc.dram_tensor("woT", [GO, D], F32R, kind="ExternalInput").ap()
    cosT = nc.dram_tensor("cosT", [P, s], F32, kind="ExternalInput").ap()
    sinT = nc.dram_tensor("sinT", [P, s], F32, kind="ExternalInput").ap()
    fnat = nc.dram_tensor("fnat", [P, 1], F32, kind="ExternalInput").ap()
    msq = nc.dram_tensor("msq", [2, P, GH], F32R, kind="ExternalInput").ap()
    mR = nc.dram_tensor("mR", [NT, GH, P], F32R, kind="ExternalInput").ap()
    mP = nc.dram_tensor("mP", [NT, 2, P, P], F32R, kind="ExternalInput").ap()
    zerod = nc.dram_tensor("zerod", [HD, CH], F32R, kind="ExternalInput").ap()
    out = nc.dram_tensor("out", [s, D], F32, kind="ExternalOutput").ap()

    with nc.allow_low_precision(reason="fp32r/bf16 rounding is intentional"), \
            tile.TileContext(nc) as tc, ExitStack() as ctx:
        consts = ctx.enter_context(tc.tile_pool(name="consts", bufs=1))
        wqpool = ctx.enter_context(tc.tile_pool(name="wqpool", bufs=1))
        wkpool = ctx.enter_context(tc.tile_pool(name="wkpool", bufs=1))
        wvpool = ctx.enter_context(tc.tile_pool(name="wvpool", bufs=1))
        wopool = ctx.enter_context(tc.tile_pool(name="wopool", bufs=1))
        xpool = ctx.enter_context(tc.tile_pool(name="xpool", bufs=8))
        cspool = ctx.enter_context(tc.tile_pool(name="cspool", bufs=1))
        qrpool = ctx.enter_context(tc.tile_pool(name="qrpool", bufs=4))
        sqpool = ctx.enter_context(tc.tile_pool(name="sqpool", bufs=4))
        rqpool = ctx.enter_context(tc.tile_pool(name="rqpool", bufs=2))
        bqpool = ctx.enter_context(tc.tile_pool(name="bqpool", bufs=1))
        qnpool = ctx.enter_context(tc.tile_pool(name="qnpool", bufs=8))
        knpool = ctx.enter_context(tc.tile_pool(name="knpool", bufs=4 * nch))
        vpool = ctx.enter_context(tc.tile_pool(name="vpool", bufs=4 * nch))
        ppool = ctx.enter_context(tc.tile_pool(name="ppool", bufs=3))
        rspool = ctx.enter_context(tc.tile_pool(name="rspool", bufs=1))
        obpool = ctx.enter_context(tc.tile_pool(name="obpool", bufs=1))
        cxpool = ctx.enter_context(tc.tile_pool(name="cxpool", bufs=4))
        psum = ctx.enter_context(tc.tile_pool(name="psum", bufs=2, space="PSUM"))

        # --- tiny constants ---
        zb = consts.tile([P, 1], F32, tag="zb", name="zb")
        nc.vector.memset(zb, 0.0)
        epsb = consts.tile([P, 1], F32, tag="epsb", name="epsb")
        nc.vector.memset(epsb, EPS)
        f_sb = consts.tile([P, 1], F32, tag="f_sb", name="f_sb")
        nc.sync.dma_start(out=f_sb, in_=fnat)

        msq_sb, mR_sb, mP_sb = [], [], []

        def load_small_consts():
            for hg in range(2):
                t_ = consts.tile([P, GH], F32, tag=f"msq{hg}", name=f"msq{hg}")
                nc.sync.dma_start(out=t_.bitcast(F32R), in_=msq[hg])
                msq_sb.append(t_)
            for t in range(NT):
                t_ = consts.tile([GH, P], F32, tag=f"mR{t}", name=f"mR{t}")
                nc.sync.dma_start(out=t_.bitcast(F32R), in_=mR[t])
                mR_sb.append(t_)
            for t in range(NT):
                row = []
                for sr in range(2):
                    t_ = consts.tile([P, P], F32, tag=f"mP{t}_{sr}",
                                     name=f"mP{t}_{sr}")
                    nc.sync.dma_start(out=t_.bitcast(F32R), in_=mP[t, sr])
                    row.append(t_)
                mP_sb.append(row)

        # --- resident weights / streamed per-chunk tiles ---
        wq_sb, wk_sb, wv_sb, wo_sb = [], [], [], []
        x_tiles = {}     # j -> list of 8 [P, CH] tiles
        cs_tiles = {}    # j -> (cos, sin)
        kn_t = {}        # (t, jc) -> [P, CH]: c-dims (2 heads) x k-positions
        qn_t = {}        # (t, j) -> [P, CH]
        v_t = {}         # s-tile -> [P, GH, HD+1] bf16 (ones col appended)

        def load_x(j):
            xt = []
            for k in range(KT):
                t_ = xpool.tile([P, CH], F32, tag="xt", name="xt")
                nc.sync.dma_start(
                    out=t_.bitcast(F32R),
                    in_=xT[k * P:(k + 1) * P, j * CH:(j + 1) * CH])
                xt.append(t_)
            x_tiles[j] = xt
            cos_sb = cspool.tile([P, CH], F32, tag="cos", name="cos")
            nc.sync.dma_start(out=cos_sb, in_=cosT[:, j * CH:(j + 1) * CH])
            sin_sb = cspool.tile([P, CH], F32, tag="sin", name="sin")
            nc.sync.dma_start(out=sin_sb, in_=sinT[:, j * CH:(j + 1) * CH])
            cs_tiles[j] = (cos_sb, sin_sb)

        def projA_steps(j, w_sb):
            """Phase A of a q/k projection as 4 filler thunks (one per psq
            slice: 8 matmuls, plus rope + squares after odd slices).  Only
            touches the 'ps' PSUM ring so the thunks are safe to emit inside
            an attention k-loop.  Returns (thunks, state)."""
            st = {"qr": [None] * NT, "sq": [None] * NT, "pair": [None, None]}

            def step(m):
                xt = x_tiles[j]
                cos_sb, sin_sb = cs_tiles[j]
                ps = psum.tile([P, CH], F32, tag="ps", name="ps")
                for k in range(KT):
                    _mm(ps, w_sb[k][:, m * P:(m + 1) * P], xt[k],
                        start=(k == 0), stop=(k == KT - 1))
                st["pair"][m % 2] = ps
                if m % 2 == 1:
                    a, b = st["pair"]
                    t1 = qrpool.tile([P, CH], F32, tag="qr", name="qr")
                    nc.vector.tensor_tensor(t1.bitcast(F32R), a, cos_sb, MULT)
                    t4 = qrpool.tile([P, CH], F32, tag="rtmp", name="rtmp",
                                     bufs=1)
                    nc.vector.tensor_tensor(t4, a, sin_sb, MULT)
                    t2 = sqpool.tile([P, CH], F32, tag="rtmp2", name="rtmp2",
                                     bufs=1)
                    nc.vector.tensor_tensor(t2, b, sin_sb, MULT)
                    t3 = qrpool.tile([P, CH], F32, tag="qr", name="qr")
                    nc.vector.tensor_tensor(t3.bitcast(F32R), b, cos_sb, MULT)
                    nc.vector.tensor_sub(t1.bitcast(F32R), t1, t2)
                    nc.vector.tensor_add(t3.bitcast(F32R), t3, t4)
                    st["qr"][m - 1], st["qr"][m] = t1, t3
                    for mq in (m - 1, m):
                        s_ = sqpool.tile([P, CH], F32, tag="sq", name="sq")
                        nc.gpsimd.tensor_mul(s_.bitcast(F32R), st["qr"][mq],
                                             st["qr"][mq])
                        st["sq"][mq] = s_

            return [lambda m=m: step(m) for m in range(NT)], st

        def phaseB(j, st, is_k):
            """RMS (ln->exp(-0.5*)) + repermute + scale for one path.
            Uses the 'pv' PSUM ring: only emit at t-loop boundaries."""
            qr, sq = st["qr"], st["sq"]
            pss = psum.tile([GH, CH], F32, tag="pv", name="pss")
            for m in range(NT):
                _mm(pss, msq_sb[m // 2], sq[m],
                    start=(m == 0), stop=(m == NT - 1))
            lnv = rqpool.tile([GH, CH], F32, tag="lnv", name="lnv", bufs=1)
            nc.scalar.activation(lnv, pss, mybir.ActivationFunctionType.Ln,
                                 bias=epsb[0:GH], scale=1.0 / HD)
            rqcR = rqpool.tile([GH, CH], F32, tag="rqR", name="rqR", bufs=2)
            nc.scalar.activation(rqcR.bitcast(F32R), lnv,
                                 mybir.ActivationFunctionType.Exp,
                                 bias=zb[0:GH], scale=-0.5)
            for t in range(NT):
                psb = psum.tile([P, CH], F32, tag="pv", name="psb")
                _mm(psb, mR_sb[t], rqcR)
                rqs = bqpool.tile([P, CH], F32, tag="bq", name="bq")
                nc.vector.tensor_copy(rqs, psb)
                psr = psum.tile([P, CH], F32, tag="pv", name="psr")
                _mm(psr, mP_sb[t][0], qr[2 * (t // 2)], start=True, stop=False)
                _mm(psr, mP_sb[t][1], qr[2 * (t // 2) + 1],
                    start=False, stop=True)
                if is_k:
                    dst = knpool.tile([P, CH], F32, tag="kn", name="kn")
                    nc.vector.scalar_tensor_tensor(
                        dst.bitcast(F32R), in0=psr, scalar=f_sb, in1=rqs,
                        op0=MULT, op1=MULT)
                    kn_t[(t, j)] = dst
                else:
                    # per-head zero-padded tiles: head data at its kn
                    # partition range, zeros elsewhere (K=128 score matmuls
                    # at full rate; K=64 row-tiled pairs measured 1.5x
                    # slower with fp32r)
                    for h2 in range(2):
                        po = HD * h2
                        qz = qnpool.tile([P, CH], F32, tag="qn", name="qn")
                        nc.sync.dma_start(
                            out=qz[HD - po:2 * HD - po, :].bitcast(F32R),
                            in_=zerod)
                        nc.vector.tensor_tensor(
                            qz[po:po + HD, :].bitcast(F32R),
                            psr[po:po + HD, :], rqs[po:po + HD, :], MULT)
                        qn_t[(2 * t + h2, j)] = qz

        def projv_steps(j):
            def step(si):
                xt = x_tiles[j]
                ps = psum.tile([P, CH], F32, tag="ps", name="ps")
                for k in range(KT):
                    _mm(ps, xt[k][:, si * P:(si + 1) * P], wv_sb[k],
                        start=(k == 0), stop=(k == KT - 1))
                vt = vpool.tile([P, GH, HD + 1], BF16, tag="vt", name="vt")
                nc.gpsimd.memset(vt[:, :, HD:HD + 1], 1.0)
                nc.scalar.copy(vt[:, :, 0:HD],
                               ps.rearrange("p (h d) -> p h d", h=GH))
                v_t[j * NT + si] = vt

            return [lambda si=si: step(si) for si in range(NT)]

        def attention(j, fillers=(), hooks=()):
            """Causal attention for chunk j's queries -> (ctx tiles, rr).

            fillers: sorted [(slot, thunk)] popped as progress (t + frac)
            passes each slot — next chunk's projection phase-A work that
            keeps the PE dense while the k-loop is exp-paced.  hooks: {t:
            thunk} run at t-loop boundaries (phase-B work on the 'pv' ring,
            which is only WAR-safe between t iterations)."""
            kmax = 4 * j + 3
            fq = list(fillers)
            hooks = dict(hooks)
            rr = rqpool.tile([GH, CH], F32, tag="rr", name="rr", bufs=1)
            ctx_t = [cxpool.tile([P, CH], F32, tag="cx", name="cx")
                     for _ in range(NT)]
            for t in range(NT):
                while fq and fq[0][0] <= t:
                    fq.pop(0)[1]()
                if t in hooks:
                    hooks[t]()
                pvs = [psum.tile([HD + 1, CH], F32, tag="pv", name="pv")
                       for _ in range(2)]
                LAG = 2
                pending = {}

                def emit_pv(kk):
                    c0k, p3k = pending.pop(kk)
                    for h2 in range(2):
                        nc.tensor.matmul(
                            pvs[h2][:, c0k:], v_t[kk][:, 2 * t + h2, :],
                            p3k[:, h2, c0k:],
                            start=(kk == 0), stop=(kk == kmax))

                for k in range(kmax + 1):
                    c0 = max(0, 128 * k - CH * j)
                    kt = kn_t[(t, k // 4)]
                    kwin = slice((k % 4) * P, (k % 4) * P + P)
                    ssp = psum.tile([P, 2, CH], F32, tag="ssp", name="ssp")
                    for h2 in range(2):
                        _mm(ssp[:, h2, c0:], kt[:, kwin],
                            qn_t[(2 * t + h2, j)][:, c0:])
                    p3 = ppool.tile([P, 2, CH], BF16, tag="pp", name="pp")
                    nc.scalar.activation(p3[:, :, c0:], ssp[:, :, c0:],
                                         mybir.ActivationFunctionType.Exp,
                                         bias=zb, scale=1.0)
                    if k >= 4 * j:
                        # in-block causal triangle: zero q < k entries
                        nc.gpsimd.affine_select(
                            out=p3[:, :, c0:c0 + P], in_=p3[:, :, c0:c0 + P],
                            pattern=[[0, 2], [1, P]],
                            compare_op=mybir.AluOpType.is_ge,
                            fill=0.0, base=0, channel_multiplier=-1)
                    pending[k] = (c0, p3)
                    if k >= LAG:
                        emit_pv(k - LAG)
                    prog = t + (k + 1.0) / (kmax + 1)
                    while fq and fq[0][0] <= prog:
                        fq.pop(0)[1]()
                for kk in range(max(0, kmax + 1 - LAG), kmax + 1):
                    emit_pv(kk)
                for h2 in range(2):
                    hl, po = 2 * t + h2, HD * h2
                    nc.vector.tensor_copy(
                        ctx_t[t][po:po + HD, :].bitcast(F32R),
                        pvs[h2][0:HD, :])
                    rs = rspool.tile([1, CH], F32, tag="rs", name="rs")
                    nc.vector.tensor_copy(rs, pvs[h2][HD:HD + 1, :])
                    nc.sync.dma_start(out=rr[hl:hl + 1, :], in_=rs)
            for _, thunk in fq:
                thunk()
            return ctx_t, rr

        def denom_recip(rr):
            rscr = rqpool.tile([GH, CH], F32, tag="lnv", name="rscr", bufs=1)
            rrc = rqpool.tile([GH, CH], F32, tag="rrc", name="rrc", bufs=1)
            nc.vector.reciprocal_approx_accurate(out=rrc, in_=rr, scratch=rscr)
            rrR = rqpool.tile([GH, CH], F32, tag="rqR", name="rrR", bufs=2)
            nc.vector.tensor_copy(rrR.bitcast(F32R), rrc)
            return rrR

        def denom_apply(ctx_t, rrR):
            for t in range(NT):
                psn = psum.tile([P, CH], F32, tag="ps", name="psn")
                _mm(psn, mR_sb[t], rrR)
                nc.vector.tensor_tensor(ctx_t[t].bitcast(F32R), psn,
                                        ctx_t[t], MULT)

        def outproj(j, ctx_t):
            for si in range(NT):
                for oc in range(2):
                    pso = psum.tile([P, CH], F32, tag="ps", name="pso")
                    for ct in range(NT):
                        _mm(pso, ctx_t[ct][:, si * P:(si + 1) * P],
                            wo_sb[ct][:, oc * CH:(oc + 1) * CH],
                            start=(ct == 0), stop=(ct == NT - 1))
                    ob = obpool.tile([P, CH], F32, tag="ob", name="ob")
                    nc.scalar.copy(ob, pso)
                    nc.sync.dma_start(
                        out=out[(j * NT + si) * P:(j * NT + si + 1) * P,
                                oc * CH:(oc + 1) * CH],
                        in_=ob)

        # ---- bootstrap: chunk 0 ----
        load_small_consts()
        # wq/x next so chunk 0's q-projection starts ASAP
        x0 = []
        for k in range(KT):
            t_ = wqpool.tile([P, GO], F32, tag=f"wq{k}", name=f"wq{k}")
            nc.sync.dma_start(out=t_.bitcast(F32R),
                              in_=wqT[k * P:(k + 1) * P, :])
            wq_sb.append(t_)
            t_ = xpool.tile([P, CH], F32, tag="xt", name="xt")
            nc.sync.dma_start(out=t_.bitcast(F32R),
                              in_=xT[k * P:(k + 1) * P, 0:CH])
            x0.append(t_)
        x_tiles[0] = x0
        cos_sb = cspool.tile([P, CH], F32, tag="cos", name="cos")
        nc.sync.dma_start(out=cos_sb, in_=cosT[:, 0:CH])
        sin_sb = cspool.tile([P, CH], F32, tag="sin", name="sin")
        nc.sync.dma_start(out=sin_sb, in_=sinT[:, 0:CH])
        cs_tiles[0] = (cos_sb, sin_sb)
        for k in range(KT):
            t_ = wkpool.tile([P, GO], F32, tag=f"wk{k}", name=f"wk{k}")
            nc.sync.dma_start(out=t_.bitcast(F32R),
                              in_=wkT[k * P:(k + 1) * P, :])
            wk_sb.append(t_)
        for k in range(KT):
            t_ = wvpool.tile([P, GO], F32, tag=f"wv{k}", name=f"wv{k}")
            nc.sync.dma_start(out=t_.bitcast(F32R),
                              in_=wvT[k * P:(k + 1) * P, :])
            wv_sb.append(t_)
        qA, qst = projA_steps(0, wq_sb)
        for th in qA:
            th()
        phaseB(0, qst, is_k=False)
        kA, kst = projA_steps(0, wk_sb)
        for th in kA:
            th()
        phaseB(0, kst, is_k=True)
        for th in projv_steps(0):
            th()
        for ct in range(NT):
            t_ = wopool.tile([P, D], F32, tag=f"wo{ct}", name=f"wo{ct}")
            nc.sync.dma_start(out=t_.bitcast(F32R),
                              in_=woT[ct * P:(ct + 1) * P, :])
            wo_sb.append(t_)

        # ---- main loop ----
        for j in range(nch):
            fillers, hooks, qst = [], {}, None
            if j + 1 < nch:
                load_x(j + 1)          # prefetch next chunk's x + cos/sin
                kA, kst = projA_steps(j + 1, wk_sb)
                qA, qst = projA_steps(j + 1, wq_sb)
                vS = projv_steps(j + 1)
                fillers = sorted(
                    [(0.0 + (i + 1) * 0.5, th) for i, th in enumerate(kA)]
                    + [(1.25 + (i + 1) * 0.5, th) for i, th in enumerate(qA)]
                    + [(2.25 + (i + 1) * 0.5, th) for i, th in enumerate(vS)],
                    key=lambda x: x[0])
                jn = j + 1
                hooks = {2: (lambda jn=jn, kst=kst:
                             phaseB(jn, kst, is_k=True))}
            ctx_t, rr = attention(j, fillers, hooks)
            rrR = denom_recip(rr)
            if qst is not None:
                phaseB(j + 1, qst, is_k=False)  # fills the recip window
            denom_apply(ctx_t, rrR)
            outproj(j, ctx_t)

    nc.compile()
    return nc


# ---------------------------------------------------------------------------
# Host-side preparation
# ---------------------------------------------------------------------------

def _softplus(x):
    return np.logaddexp(0.0, x)


def _host_tables(s, q_ln_scale, k_ln_scale, per_dim_scale):
    pos = np.arange(s, dtype=np.float64)
    i = np.arange(HALF, dtype=np.float64)
    timescale = 10000.0 ** (2.0 * i / HD)
    ang = pos[None, :] / timescale[:, None]          # [32, s]
    cosT = np.tile(np.cos(ang), (4, 1)).astype(np.float32)  # [128, s]
    sinT = np.tile(np.sin(ang), (4, 1)).astype(np.float32)

    hd = np.arange(P) % HD
    f = (q_ln_scale[hd] * k_ln_scale[hd]
         * (LOG2_E / math.sqrt(HD))
         * _softplus(per_dim_scale[hd].astype(np.float64))).astype(np.float32)
    fnat = f.reshape(P, 1)

    # o2' layout: slice m of the 512 group dims holds half b=m%2 of heads
    # 4*(m//2)..4*(m//2)+3; within a slice, r = 32*hl_local + i.
    msq = np.zeros((2, P, GH), np.float32)
    for hg in range(2):
        for r in range(P):
            msq[hg, r, 4 * hg + r // HALF] = 1.0

    mR = np.zeros((NT, GH, P), np.float32)
    for t in range(NT):
        for m in range(P):
            mR[t, (128 * t + m) // HD, m] = 1.0

    mP = np.zeros((NT, 2, P, P), np.float32)
    for t in range(NT):
        for p in range(P):
            n = 128 * t + p
            hl, d = n // HD, n % HD
            b, i_ = d // HALF, d % HALF
            mP[t, b, 32 * (hl % 4) + i_, p] = 1.0

    return cosT, sinT, fnat, msq, mR, mP


def _oprime_perm():
    """o2'[j] -> natural local dim, for one head group (512 dims)."""
    perm = np.zeros(GO, np.int64)
    for j in range(GO):
        sl, r = j // 128, j % 128
        b, hg = sl % 2, sl // 2
        hl, i = 4 * hg + r // HALF, r % HALF
        perm[j] = HD * hl + HALF * b + i
    return perm


def _numpy_reference(inputs_q, Wq, Wk, Wv, Wo, q_ln_scale, k_ln_scale,
                     per_dim_scale, patch_mask):
    """Exact numpy replica of the reference (fallback for patch_mask != 0)."""
    b, s, d = inputs_q.shape
    x = inputs_q.astype(np.float32)
    q = (x @ Wq.T).reshape(b, s, H, HD)
    k = (x @ Wk.T).reshape(b, s, H, HD)
    v = (x @ Wv.T).reshape(b, s, H, HD)
    num_masked = patch_mask.astype(np.int64).sum(-1)
    position = np.arange(s)[None, :] - num_masked[:, None]

    def rope(t):
        frac = 2.0 * np.arange(HALF) / HD
        ts = 10000.0 ** frac
        ang = position[:, :, None, None].astype(np.float32) / ts[None, None, None, :]
        sin, cos = np.sin(ang), np.cos(ang)
        f, sec = t[..., :HALF], t[..., HALF:]
        return np.concatenate([f * cos - sec * sin, sec * cos + f * sin], -1)

    def rms(t, scale):
        var = np.mean(np.square(t), -1, keepdims=True)
        return t / np.sqrt(var + EPS) * scale

    q = rms(rope(q), q_ln_scale)
    k = rms(rope(k), k_ln_scale)
    q = q * (LOG2_E / math.sqrt(HD) * _softplus(per_dim_scale)).astype(np.float32)
    scores = np.einsum("bqhd,bkhd->bhqk", q, k)
    qi = np.arange(s)[None, None, :, None]
    ki = np.arange(s)[None, None, None, :]
    mask = (qi >= ki) & (ki >= num_masked[:, None, None, None])
    neg = -np.finfo(np.float32).max / 2
    scores = np.where(mask, scores, neg)
    scores = scores - scores.max(-1, keepdims=True)
    e = np.exp(scores)
    attn = e / e.sum(-1, keepdims=True)
    o = np.einsum("bhqk,bkhd->bqhd", attn, v).reshape(b, s, d)
    return (o @ Wo.T).astype(np.float32)


_NC_CACHE = {}


def _get_nc(s):
    if s not in _NC_CACHE:
        _NC_CACHE[s] = build_bass(s)
    return _NC_CACHE[s]


def make_in_maps(inputs_q, Wq, Wk, Wv, Wo, q_ln_scale, k_ln_scale,
                 per_dim_scale, s):
    cosT, sinT, fnat, msq, mR, mP = _host_tables(
        s, np.asarray(q_ln_scale, np.float32),
        np.asarray(k_ln_scale, np.float32),
        np.asarray(per_dim_scale, np.float32))
    perm = _oprime_perm()

    xT = [_round_fp32r(np.asarray(inputs_q[b], np.float32).T)
          for b in range(inputs_q.shape[0])]
    wq_g, wk_g, wv_g, wo_g = [], [], [], []
    for g in range(2):
        rows = g * GO + perm
        wq_g.append(_round_fp32r(np.asarray(Wq, np.float32)[rows, :].T))
        wk_g.append(_round_fp32r(np.asarray(Wk, np.float32)[rows, :].T))
        sl = slice(g * GO, (g + 1) * GO)
        wv_g.append(_round_fp32r(np.asarray(Wv, np.float32)[sl, :].T))
        wo_g.append(_round_fp32r(np.asarray(Wo, np.float32)[:, sl].T))

    in_maps = []
    for c in range(N_CORES):
        b, g = (c // 2) % len(xT), c % 2
        in_maps.append({
            "xT": xT[b], "wqT": wq_g[g], "wkT": wk_g[g], "wvT": wv_g[g],
            "woT": wo_g[g], "cosT": cosT, "sinT": sinT, "fnat": fnat,
            "msq": msq, "mR": mR, "mP": mP,
            "zerod": np.zeros((HD, CH), np.float32),
        })
    return in_maps


def kernel(inputs_q, Wq, Wk, Wv, Wo, q_ln_scale, k_ln_scale,
           per_dim_scale, patch_mask):
    global LAST_RESULTS
    inputs_q = np.asarray(inputs_q, np.float32)
    patch_mask = np.asarray(patch_mask)
    if patch_mask.astype(np.int64).sum() != 0:
        return _numpy_reference(
            inputs_q, np.asarray(Wq, np.float32), np.asarray(Wk, np.float32),
            np.asarray(Wv, np.float32), np.asarray(Wo, np.float32),
            np.asarray(q_ln_scale, np.float32),
            np.asarray(k_ln_scale, np.float32),
            np.asarray(per_dim_scale, np.float32), patch_mask)

    s = inputs_q.shape[1]
    in_maps = make_in_maps(inputs_q, Wq, Wk, Wv, Wo, q_ln_scale, k_ln_scale,
                           per_dim_scale, s)
    nc = _get_nc(s)
    res = run_bass_kernel_spmd(
        nc, in_maps, core_ids=list(range(N_CORES)),
        trace=bool(os.environ.get("KERNEL_TRACE")),
        tmpdir=os.environ.get("KERNEL_TMPDIR") or None,
    )
    LAST_RESULTS = res
    outs = [r["out"] for r in res.results]
    full = np.empty((inputs_q.shape[0], s, D), np.float32)
    for b in range(inputs_q.shape[0]):
        full[b] = outs[2 * b] + outs[2 * b + 1]
    return full


# revision 26
# speedup vs baseline: 1.0983x; 1.0795x over previous
"""Trainium2 Bass kernel: multi-head attention (B=4, S=2048, D=1024, H=16, HD=64).

Sharding: 8 cores = 4 batches x 2 head-groups (8 heads each).
Each core computes, for its (batch b, head-group g):
    qT/kT (RoPE'd, RMS-normed, scale-folded) via projections with
    host-pre-transposed inputs/weights, v in natural layout (bf16),
    causal flash-style attention (no max subtraction; fp32 range is
    ample), and a partial output projection with the group's Wo rows.
Host sums the two partial outputs per batch.

v2 schedule: resident weights, K=64 row-tiled score pairs, h2-batched
exp into 2-bank PSUM tiles, affine_select triangle masks, ln/exp rms
(single ACT table set), and proj(j+1) emission inside chunk j's
softmax-denominator window with outproj/proj_v as rms-stall fillers.
"""

import math
import os
from contextlib import ExitStack

import numpy as np

import concourse.bacc as bacc
import concourse.bass as bass
import concourse.mybir as mybir
import concourse.tile as tile
from concourse.bass_utils import run_bass_kernel_spmd

try:
    from neuron_dtypes._impl.fp32r import cast_fp32_to_fp32r as _c32r
except Exception:  # pragma: no cover
    _c32r = None


def _round_fp32r(a):
    """Round fp32 array to the fp32r encoding the PE consumes (TF32-like)."""
    a = np.ascontiguousarray(a, np.float32)
    if _c32r is None:
        u = a.view(np.uint32)
        low = u & 0xFFF
        u = (u & ~np.uint32(0xFFF)) + np.where(
            (low > 0x800) | ((low == 0x800) & ((u >> 12) & 1).astype(bool)),
            np.uint32(0x1000), np.uint32(0))
        return u.view(np.float32)
    flat = a.reshape(-1).view(np.uint32)
    out = _c32r(flat.size, flat)
    return np.asarray(out, np.uint32).reshape(a.shape).view(np.float32)

B, D, H, HD = 4, 1024, 16, 64
S_FULL = 2048
HALF = 32          # rope pair offset within a head
GH = 8             # heads per core (head-group)
GO = GH * HD       # 512 projection dims per group
EPS = 1e-6
LOG2_E = 1.442695041
N_CORES = 8
P = 128            # partitions
CH = 512           # s-chunk width (matmul free dim)
KT = D // P        # 8 contraction tiles
NT = GO // P       # 4 partition tiles of the group's 512 dims
F32 = mybir.dt.float32
F32R = mybir.dt.float32r
BF16 = mybir.dt.bfloat16
MULT = mybir.AluOpType.mult

LAST_RESULTS = None  # BassKernelResults of the most recent run (for profiling)


def build_bass(s=S_FULL):
    nch = s // CH          # s-chunks

    nc = bacc.Bacc("TRN2", target_bir_lowering=False, debug=False)

    def _mm(out, lhsT, rhs, start=True, stop=True):
        nc.tensor.matmul(
            out, lhsT.bitcast(F32R), rhs.bitcast(F32R), start=start, stop=stop
        )

    xT = nc.dram_tensor("xT", [D, s], F32R, kind="ExternalInput").ap()
    wqT = nc.dram_tensor("wqT", [D, GO], F32R, kind="ExternalInput").ap()
    wkT = nc.dram_tensor("wkT", [D, GO], F32R, kind="ExternalInput").ap()
    wvT = nc.dram_tensor("wvT", [D, GO], F32R, kind="ExternalInput").ap()
    woT = nc.dram_tensor("woT", [GO, D], F32R, kind="ExternalInput").ap()
    cosT = nc.dram_tensor("cosT", [P, s], F32, kind="ExternalInput").ap()
    sinT = nc.dram_tensor("sinT", [P, s], F32, kind="ExternalInput").ap()
    fnat = nc.dram_tensor("fnat", [P, 1], F32, kind="ExternalInput").ap()
    msq = nc.dram_tensor("msq", [2, P, GH], F32R, kind="ExternalInput").ap()
    mR = nc.dram_tensor("mR", [NT, GH, P], F32R, kind="ExternalInput").ap()
    mP = nc.dram_tensor("mP", [NT, 2, P, P], F32R, kind="ExternalInput").ap()
    zerod = nc.dram_tensor("zerod", [HD, CH], F32R, kind="ExternalInput").ap()
    out = nc.dram_tensor("out", [s, D], F32, kind="ExternalOutput").ap()

    with nc.allow_low_precision(reason="fp32r/bf16 rounding is intentional"), \
            tile.TileContext(nc) as tc, ExitStack() as ctx:
        consts = ctx.enter_context(tc.tile_pool(name="consts", bufs=1))
        wqpool = ctx.enter_context(tc.tile_pool(name="wqpool", bufs=1))
        wkpool = ctx.enter_context(tc.tile_pool(name="wkpool", bufs=1))
        wvpool = ctx.enter_context(tc.tile_pool(name="wvpool", bufs=1))
        wopool = ctx.enter_context(tc.tile_pool(name="wopool", bufs=1))
        xpool = ctx.enter_context(tc.tile_pool(name="xpool", bufs=8))
        cspool = ctx.enter_context(tc.tile_pool(name="cspool", bufs=1))
        qrpool = ctx.enter_context(tc.tile_pool(name="qrpool", bufs=4))
        sqpool = ctx.enter_context(tc.tile_pool(name="sqpool", bufs=4))
        rqpool = ctx.enter_context(tc.tile_pool(name="rqpool", bufs=2))
        bqpool = ctx.enter_context(tc.tile_pool(name="bqpool", bufs=1))
        qnpool = ctx.enter_context(tc.tile_pool(name="qnpool", bufs=8))
        knpool = ctx.enter_context(tc.tile_pool(name="knpool", bufs=4 * nch))
        vpool = ctx.enter_context(tc.tile_pool(name="vpool", bufs=4 * nch))
        ppool = ctx.enter_context(tc.tile_pool(name="ppool", bufs=4))
        rspool = ctx.enter_context(tc.tile_pool(name="rspool", bufs=1))
        obpool = ctx.enter_context(tc.tile_pool(name="obpool", bufs=1))
        cxpool = ctx.enter_context(tc.tile_pool(name="cxpool", bufs=4))
        psum = ctx.enter_context(tc.tile_pool(name="psum", bufs=2, space="PSUM"))

        # --- tiny constants ---
        zb = consts.tile([P, 1], F32, tag="zb", name="zb")
        nc.vector.memset(zb, 0.0)
        epsb = consts.tile([P, 1], F32, tag="epsb", name="epsb")
        nc.vector.memset(epsb, EPS)
        f_sb = consts.tile([P, 1], F32, tag="f_sb", name="f_sb")
        nc.sync.dma_start(out=f_sb, in_=fnat)

        msq_sb, mR_sb, mP_sb = [], [], []

        def load_small_consts():
            for hg in range(2):
                t_ = consts.tile([P, GH], F32, tag=f"msq{hg}", name=f"msq{hg}")
                nc.sync.dma_start(out=t_.bitcast(F32R), in_=msq[hg])
                msq_sb.append(t_)
            for t in range(NT):
                t_ = consts.tile([GH, P], F32, tag=f"mR{t}", name=f"mR{t}")
                nc.sync.dma_start(out=t_.bitcast(F32R), in_=mR[t])
                mR_sb.append(t_)
            for t in range(NT):
                row = []
                for sr in range(2):
                    t_ = consts.tile([P, P], F32, tag=f"mP{t}_{sr}",
                                     name=f"mP{t}_{sr}")
                    nc.sync.dma_start(out=t_.bitcast(F32R), in_=mP[t, sr])
                    row.append(t_)
                mP_sb.append(row)

        # --- resident weights / streamed per-chunk tiles ---
        wq_sb, wk_sb, wv_sb, wo_sb = [], [], [], []
        x_tiles = {}     # j -> list of 8 [P, CH] tiles
        cs_tiles = {}    # j -> (cos, sin)
        kn_t = {}        # (t, jc) -> [P, CH]: c-dims (2 heads) x k-positions
        qn_t = {}        # (t, j) -> [P, CH]
        v_t = {}         # s-tile -> [P, GH, HD+1] bf16 (ones col appended)

        def load_x(j):
            xt = []
            for k in range(KT):
                t_ = xpool.tile([P, CH], F32, tag="xt", name="xt")
                nc.sync.dma_start(
                    out=t_.bitcast(F32R),
                    in_=xT[k * P:(k + 1) * P, j * CH:(j + 1) * CH])
                xt.append(t_)
            x_tiles[j] = xt
            cos_sb = cspool.tile([P, CH], F32, tag="cos", name="cos")
            nc.sync.dma_start(out=cos_sb, in_=cosT[:, j * CH:(j + 1) * CH])
            sin_sb = cspool.tile([P, CH], F32, tag="sin", name="sin")
            nc.sync.dma_start(out=sin_sb, in_=sinT[:, j * CH:(j + 1) * CH])
            cs_tiles[j] = (cos_sb, sin_sb)

        def projA_steps(j, w_sb):
            """Phase A of a q/k projection as 4 filler thunks (one per psq
            slice: 8 matmuls, plus rope + squares after odd slices).  Only
            touches the 'ps' PSUM ring so the thunks are safe to emit inside
            an attention k-loop.  Returns (thunks, state)."""
            st = {"qr": [None] * NT, "sq": [None] * NT, "pair": [None, None]}

            def step(m):
                xt = x_tiles[j]
                cos_sb, sin_sb = cs_tiles[j]
                ps = psum.tile([P, CH], F32, tag="ps", name="ps")
                for k in range(KT):
                    _mm(ps, w_sb[k][:, m * P:(m + 1) * P], xt[k],
                        start=(k == 0), stop=(k == KT - 1))
                st["pair"][m % 2] = ps
                if m % 2 == 1:
                    a, b = st["pair"]
                    t1 = qrpool.tile([P, CH], F32, tag="qr", name="qr")
                    nc.vector.tensor_tensor(t1.bitcast(F32R), a, cos_sb, MULT)
                    t4 = qrpool.tile([P, CH], F32, tag="rtmp", name="rtmp",
                                     bufs=1)
                    nc.vector.tensor_tensor(t4, a, sin_sb, MULT)
                    t2 = sqpool.tile([P, CH], F32, tag="rtmp2", name="rtmp2",
                                     bufs=1)
                    nc.vector.tensor_tensor(t2, b, sin_sb, MULT)
                    t3 = qrpool.tile([P, CH], F32, tag="qr", name="qr")
                    nc.vector.tensor_tensor(t3.bitcast(F32R), b, cos_sb, MULT)
                    nc.vector.tensor_sub(t1.bitcast(F32R), t1, t2)
                    nc.vector.tensor_add(t3.bitcast(F32R), t3, t4)
                    st["qr"][m - 1], st["qr"][m] = t1, t3
                    for mq in (m - 1, m):
                        s_ = sqpool.tile([P, CH], F32, tag="sq", name="sq")
                        nc.gpsimd.tensor_mul(s_.bitcast(F32R), st["qr"][mq],
                                             st["qr"][mq])
                        st["sq"][mq] = s_

            return [lambda m=m: step(m) for m in range(NT)], st

        def phaseB_pss(st):
            """Mean-square matmuls ('pv' PSUM ring: t-boundaries/drain only)."""
            pss = psum.tile([GH, CH], F32, tag="pv", name="pss")
            for m in range(NT):
                _mm(pss, msq_sb[m // 2], st["sq"][m],
                    start=(m == 0), stop=(m == NT - 1))
            return pss

        def phaseB_lnexp(pss):
            """rms scale 1/sqrt(mean_sq+eps) = exp(-0.5*ln(.)) on ACT."""
            lnv = rqpool.tile([GH, CH], F32, tag="lnv", name="lnv", bufs=1)
            nc.scalar.activation(lnv, pss, mybir.ActivationFunctionType.Ln,
                                 bias=epsb[0:GH], scale=1.0 / HD)
            rqcR = rqpool.tile([GH, CH], F32, tag="rqR", name="rqR", bufs=2)
            nc.scalar.activation(rqcR.bitcast(F32R), lnv,
                                 mybir.ActivationFunctionType.Exp,
                                 bias=zb[0:GH], scale=-0.5)
            return rqcR

        def phaseB_apply(j, st, rqcR, is_k):
            """Repermute to natural head order + apply rms scale (pv ring)."""
            qr = st["qr"]
            for t in range(NT):
                psb = psum.tile([P, CH], F32, tag="pv", name="psb")
                _mm(psb, mR_sb[t], rqcR)
                rqs = bqpool.tile([P, CH], F32, tag="bq", name="bq")
                nc.vector.tensor_copy(rqs, psb)
                psr = psum.tile([P, CH], F32, tag="pv", name="psr")
                _mm(psr, mP_sb[t][0], qr[2 * (t // 2)], start=True, stop=False)
                _mm(psr, mP_sb[t][1], qr[2 * (t // 2) + 1],
                    start=False, stop=True)
                if is_k:
                    dst = knpool.tile([P, CH], F32, tag="kn", name="kn")
                    nc.vector.scalar_tensor_tensor(
                        dst.bitcast(F32R), in0=psr, scalar=f_sb, in1=rqs,
                        op0=MULT, op1=MULT)
                    kn_t[(t, j)] = dst
                else:
                    # per-head zero-padded tiles: head data at its kn
                    # partition range, zeros elsewhere (K=128 score matmuls
                    # at full rate; K=64 row-tiled pairs measured 1.5x
                    # slower with fp32r)
                    for h2 in range(2):
                        po = HD * h2
                        qz = qnpool.tile([P, CH], F32, tag="qn", name="qn")
                        nc.sync.dma_start(
                            out=qz[HD - po:2 * HD - po, :].bitcast(F32R),
                            in_=zerod)
                        nc.vector.tensor_tensor(
                            qz[po:po + HD, :].bitcast(F32R),
                            psr[po:po + HD, :], rqs[po:po + HD, :], MULT)
                        qn_t[(2 * t + h2, j)] = qz

        def phaseB(j, st, is_k):
            phaseB_apply(j, st, phaseB_lnexp(phaseB_pss(st)), is_k)

        def projv_steps(j):
            def step(si):
                xt = x_tiles[j]
                ps = psum.tile([P, CH], F32, tag="ps", name="ps")
                for k in range(KT):
                    _mm(ps, xt[k][:, si * P:(si + 1) * P], wv_sb[k],
                        start=(k == 0), stop=(k == KT - 1))
                vt = vpool.tile([P, GH, HD + 1], BF16, tag="vt", name="vt")
                nc.gpsimd.memset(vt[:, :, HD:HD + 1], 1.0)
                nc.scalar.copy(vt[:, :, 0:HD],
                               ps.rearrange("p (h d) -> p h d", h=GH))
                v_t[j * NT + si] = vt

            return [lambda si=si: step(si) for si in range(NT)]

        def attention(j, fillers=(), hooks=()):
            """Causal attention for chunk j's queries -> (ctx tiles, rr).

            fillers: sorted [(slot, thunk)] popped as progress (t + frac)
            passes each slot — next chunk's projection phase-A work that
            keeps the PE dense while the k-loop is exp-paced.  hooks: {t:
            thunk} run at t-loop boundaries (phase-B work on the 'pv' ring,
            which is only WAR-safe between t iterations)."""
            kmax = 4 * j + 3
            fq = list(fillers)
            hooks = dict(hooks)
            rr = rqpool.tile([GH, CH], F32, tag="rr", name="rr", bufs=1)
            ctx_t = [cxpool.tile([P, CH], F32, tag="cx", name="cx")
                     for _ in range(NT)]
            for t in range(NT):
                while fq and fq[0][0] <= t:
                    fq.pop(0)[1]()
                if t in hooks:
                    def _popper(n):
                        for _ in range(n):
                            if fq:
                                fq.pop(0)[1]()
                    hooks[t](_popper)
                pvs = [psum.tile([HD + 1, CH], F32, tag="pv", name="pv")
                       for _ in range(2)]
                LAG = 3
                pending = {}

                def emit_pv(kk):
                    c0k, p3k = pending.pop(kk)
                    for h2 in range(2):
                        nc.tensor.matmul(
                            pvs[h2][:, c0k:], v_t[kk][:, 2 * t + h2, :],
                            p3k[:, h2, c0k:],
                            start=(kk == 0), stop=(kk == kmax))

                for k in range(kmax + 1):
                    c0 = max(0, 128 * k - CH * j)
                    kt = kn_t[(t, k // 4)]
                    kwin = slice((k % 4) * P, (k % 4) * P + P)
                    ssp = psum.tile([P, 2, CH], F32, tag="ssp", name="ssp")
                    for h2 in range(2):
                        _mm(ssp[:, h2, c0:], kt[:, kwin],
                            qn_t[(2 * t + h2, j)][:, c0:])
                    p3 = ppool.tile([P, 2, CH], BF16, tag="pp", name="pp")
                    nc.scalar.activation(p3[:, :, c0:], ssp[:, :, c0:],
                                         mybir.ActivationFunctionType.Exp,
                                         bias=zb, scale=1.0)
                    if k >= 4 * j:
                        # in-block causal triangle: zero q < k entries
                        nc.gpsimd.affine_select(
                            out=p3[:, :, c0:c0 + P], in_=p3[:, :, c0:c0 + P],
                            pattern=[[0, 2], [1, P]],
                            compare_op=mybir.AluOpType.is_ge,
                            fill=0.0, base=0, channel_multiplier=-1)
                    pending[k] = (c0, p3)
                    if k >= LAG:
                        emit_pv(k - LAG)
                    prog = t + (k + 1.0) / (kmax + 1)
                    while fq and fq[0][0] <= prog:
                        fq.pop(0)[1]()
                for kk in range(max(0, kmax + 1 - LAG), kmax + 1):
                    emit_pv(kk)
                for h2 in range(2):
                    hl, po = 2 * t + h2, HD * h2
                    nc.vector.tensor_copy(
                        ctx_t[t][po:po + HD, :].bitcast(F32R),
                        pvs[h2][0:HD, :])
                    rs = rspool.tile([1, CH], F32, tag="rs", name="rs")
                    nc.vector.tensor_copy(rs, pvs[h2][HD:HD + 1, :])
                    nc.sync.dma_start(out=rr[hl:hl + 1, :], in_=rs)
            for _, thunk in fq:
                thunk()
            return ctx_t, rr

        def denom_recip(rr):
            rscr = rqpool.tile([GH, CH], F32, tag="lnv", name="rscr", bufs=1)
            rrc = rqpool.tile([GH, CH], F32, tag="rrc", name="rrc", bufs=1)
            nc.vector.reciprocal_approx_accurate(out=rrc, in_=rr, scratch=rscr)
            rrR = rqpool.tile([GH, CH], F32, tag="rqR", name="rrR", bufs=2)
            nc.vector.tensor_copy(rrR.bitcast(F32R), rrc)
            return rrR

        def denom_apply(ctx_t, rrR):
            for t in range(NT):
                psn = psum.tile([P, CH], F32, tag="ps", name="psn")
                _mm(psn, mR_sb[t], rrR)
                nc.vector.tensor_tensor(ctx_t[t].bitcast(F32R), psn,
                                        ctx_t[t], MULT)

        def outproj(j, ctx_t):
            for si in range(NT):
                for oc in range(2):
                    pso = psum.tile([P, CH], F32, tag="ps", name="pso")
                    for ct in range(NT):
                        _mm(pso, ctx_t[ct][:, si * P:(si + 1) * P],
                            wo_sb[ct][:, oc * CH:(oc + 1) * CH],
                            start=(ct == 0), stop=(ct == NT - 1))
                    # alternate staging between the ob slot and the
                    # (drain-idle) rs slot: free double-buffering so the
                    # copy->DMA chain stops gating the matmul stream
                    if (si * 2 + oc) % 2 == 0:
                        ob = obpool.tile([P, CH], F32, tag="ob", name="ob")
                    else:
                        ob = rspool.tile([P, CH], F32, tag="rs", name="ob2")
                    nc.scalar.copy(ob, pso)
                    nc.sync.dma_start(
                        out=out[(j * NT + si) * P:(j * NT + si + 1) * P,
                                oc * CH:(oc + 1) * CH],
                        in_=ob)

        # ---- bootstrap: chunk 0 ----
        load_small_consts()
        # wq/x next so chunk 0's q-projection starts ASAP
        x0 = []
        for k in range(KT):
            t_ = wqpool.tile([P, GO], F32, tag=f"wq{k}", name=f"wq{k}")
            nc.sync.dma_start(out=t_.bitcast(F32R),
                              in_=wqT[k * P:(k + 1) * P, :])
            wq_sb.append(t_)
            t_ = xpool.tile([P, CH], F32, tag="xt", name="xt")
            nc.sync.dma_start(out=t_.bitcast(F32R),
                              in_=xT[k * P:(k + 1) * P, 0:CH])
            x0.append(t_)
        x_tiles[0] = x0
        cos_sb = cspool.tile([P, CH], F32, tag="cos", name="cos")
        nc.sync.dma_start(out=cos_sb, in_=cosT[:, 0:CH])
        sin_sb = cspool.tile([P, CH], F32, tag="sin", name="sin")
        nc.sync.dma_start(out=sin_sb, in_=sinT[:, 0:CH])
        cs_tiles[0] = (cos_sb, sin_sb)
        for k in range(KT):
            t_ = wkpool.tile([P, GO], F32, tag=f"wk{k}", name=f"wk{k}")
            nc.sync.dma_start(out=t_.bitcast(F32R),
                              in_=wkT[k * P:(k + 1) * P, :])
            wk_sb.append(t_)
        for k in range(KT):
            t_ = wvpool.tile([P, GO], F32, tag=f"wv{k}", name=f"wv{k}")
            nc.sync.dma_start(out=t_.bitcast(F32R),
                              in_=wvT[k * P:(k + 1) * P, :])
            wv_sb.append(t_)
        qA, qst = projA_steps(0, wq_sb)
        for th in qA:
            th()
        phaseB(0, qst, is_k=False)
        kA, kst = projA_steps(0, wk_sb)
        for th in kA:
            th()
        phaseB(0, kst, is_k=True)
        for th in projv_steps(0):
            th()
        for ct in range(NT):
            t_ = wopool.tile([P, D], F32, tag=f"wo{ct}", name=f"wo{ct}")
            nc.sync.dma_start(out=t_.bitcast(F32R),
                              in_=woT[ct * P:(ct + 1) * P, :])
            wo_sb.append(t_)

        # ---- main loop ----
        for j in range(nch):
            fillers, hooks, qst = [], {}, None
            if j + 1 < nch:
                load_x(j + 1)          # prefetch next chunk's x + cos/sin
                kA, kst = projA_steps(j + 1, wk_sb)
                qA, qst = projA_steps(j + 1, wq_sb)
                vS = projv_steps(j + 1)
                fillers = sorted(
                    [(0.0 + (i + 1) * 0.5, th) for i, th in enumerate(kA)]
                    + [(1.25 + (i + 1) * 0.5, th) for i, th in enumerate(qA)]
                    + [(2.25 + (i + 1) * 0.5, th) for i, th in enumerate(vS)],
                    key=lambda x: x[0])
                jn = j + 1

                def hook_k(popper, jn=jn, kst=kst):
                    # pss first so the rms ACT chain starts ASAP; the
                    # popped fillers then feed the PE through its two
                    # table loads
                    pss = phaseB_pss(kst)
                    popper(2)
                    rqcR = phaseB_lnexp(pss)
                    phaseB_apply(jn, kst, rqcR, is_k=True)

                hooks = {2: hook_k}
            ctx_t, rr = attention(j, fillers, hooks)
            # drain: pss matmuls fill the reciprocal window, outproj's
            # matmul stream hides the rms ACT table-load chain, and the
            # psb/psr matmuls land after it
            pss_q = phaseB_pss(qst) if qst is not None else None
            rrR = denom_recip(rr)
            denom_apply(ctx_t, rrR)
            outproj(j, ctx_t)
            if qst is not None:
                phaseB_apply(j + 1, qst, phaseB_lnexp(pss_q), is_k=False)

    nc.compile()
    return nc


# ---------------------------------------------------------------------------
# Host-side preparation
# ---------------------------------------------------------------------------

def _softplus(x):
    return np.logaddexp(0.0, x)


def _host_tables(s, q_ln_scale, k_ln_scale, per_dim_scale):
    pos = np.arange(s, dtype=np.float64)
    i = np.arange(HALF, dtype=np.float64)
    timescale = 10000.0 ** (2.0 * i / HD)
    ang = pos[None, :] / timescale[:, None]          # [32, s]
    cosT = np.tile(np.cos(ang), (4, 1)).astype(np.float32)  # [128, s]
    sinT = np.tile(np.sin(ang), (4, 1)).astype(np.float32)

    hd = np.arange(P) % HD
    f = (q_ln_scale[hd] * k_ln_scale[hd]
         * (LOG2_E / math.sqrt(HD))
         * _softplus(per_dim_scale[hd].astype(np.float64))).astype(np.float32)
    fnat = f.reshape(P, 1)

    # o2' layout: slice m of the 512 group dims holds half b=m%2 of heads
    # 4*(m//2)..4*(m//2)+3; within a slice, r = 32*hl_local + i.
    msq = np.zeros((2, P, GH), np.float32)
    for hg in range(2):
        for r in range(P):
            msq[hg, r, 4 * hg + r // HALF] = 1.0

    mR = np.zeros((NT, GH, P), np.float32)
    for t in range(NT):
        for m in range(P):
            mR[t, (128 * t + m) // HD, m] = 1.0

    mP = np.zeros((NT, 2, P, P), np.float32)
    for t in range(NT):
        for p in range(P):
            n = 128 * t + p
            hl, d = n // HD, n % HD
            b, i_ = d // HALF, d % HALF
            mP[t, b, 32 * (hl % 4) + i_, p] = 1.0

    return cosT, sinT, fnat, msq, mR, mP


def _oprime_perm():
    """o2'[j] -> natural local dim, for one head group (512 dims)."""
    perm = np.zeros(GO, np.int64)
    for j in range(GO):
        sl, r = j // 128, j % 128
        b, hg = sl % 2, sl // 2
        hl, i = 4 * hg + r // HALF, r % HALF
        perm[j] = HD * hl + HALF * b + i
    return perm


def _numpy_reference(inputs_q, Wq, Wk, Wv, Wo, q_ln_scale, k_ln_scale,
                     per_dim_scale, patch_mask):
    """Exact numpy replica of the reference (fallback for patch_mask != 0)."""
    b, s, d = inputs_q.shape
    x = inputs_q.astype(np.float32)
    q = (x @ Wq.T).reshape(b, s, H, HD)
    k = (x @ Wk.T).reshape(b, s, H, HD)
    v = (x @ Wv.T).reshape(b, s, H, HD)
    num_masked = patch_mask.astype(np.int64).sum(-1)
    position = np.arange(s)[None, :] - num_masked[:, None]

    def rope(t):
        frac = 2.0 * np.arange(HALF) / HD
        ts = 10000.0 ** frac
        ang = position[:, :, None, None].astype(np.float32) / ts[None, None, None, :]
        sin, cos = np.sin(ang), np.cos(ang)
        f, sec = t[..., :HALF], t[..., HALF:]
        return np.concatenate([f * cos - sec * sin, sec * cos + f * sin], -1)

    def rms(t, scale):
        var = np.mean(np.square(t), -1, keepdims=True)
        return t / np.sqrt(var + EPS) * scale

    q = rms(rope(q), q_ln_scale)
    k = rms(rope(k), k_ln_scale)
    q = q * (LOG2_E / math.sqrt(HD) * _softplus(per_dim_scale)).astype(np.float32)
    scores = np.einsum("bqhd,bkhd->bhqk", q, k)
    qi = np.arange(s)[None, None, :, None]
    ki = np.arange(s)[None, None, None, :]
    mask = (qi >= ki) & (ki >= num_masked[:, None, None, None])
    neg = -np.finfo(np.float32).max / 2
    scores = np.where(mask, scores, neg)
    scores = scores - scores.max(-1, keepdims=True)
    e = np.exp(scores)
    attn = e / e.sum(-1, keepdims=True)
    o = np.einsum("bhqk,bkhd->bqhd", attn, v).reshape(b, s, d)
    return (o @ Wo.T).astype(np.float32)


_NC_CACHE = {}


def _get_nc(s):
    if s not in _NC_CACHE:
        _NC_CACHE[s] = build_bass(s)
    return _NC_CACHE[s]


def make_in_maps(inputs_q, Wq, Wk, Wv, Wo, q_ln_scale, k_ln_scale,
                 per_dim_scale, s):
    cosT, sinT, fnat, msq, mR, mP = _host_tables(
        s, np.asarray(q_ln_scale, np.float32),
        np.asarray(k_ln_scale, np.float32),
        np.asarray(per_dim_scale, np.float32))
    perm = _oprime_perm()

    xT = [_round_fp32r(np.asarray(inputs_q[b], np.float32).T)
          for b in range(inputs_q.shape[0])]
    wq_g, wk_g, wv_g, wo_g = [], [], [], []
    for g in range(2):
        rows = g * GO + perm
        wq_g.append(_round_fp32r(np.asarray(Wq, np.float32)[rows, :].T))
        wk_g.append(_round_fp32r(np.asarray(Wk, np.float32)[rows, :].T))
        sl = slice(g * GO, (g + 1) * GO)
        wv_g.append(_round_fp32r(np.asarray(Wv, np.float32)[sl, :].T))
        wo_g.append(_round_fp32r(np.asarray(Wo, np.float32)[:, sl].T))

    in_maps = []
    for c in range(N_CORES):
        b, g = (c // 2) % len(xT), c % 2
        in_maps.append({
            "xT": xT[b], "wqT": wq_g[g], "wkT": wk_g[g], "wvT": wv_g[g],
            "woT": wo_g[g], "cosT": cosT, "sinT": sinT, "fnat": fnat,
            "msq": msq, "mR": mR, "mP": mP,
            "zerod": np.zeros((HD, CH), np.float32),
        })
    return in_maps


def kernel(inputs_q, Wq, Wk, Wv, Wo, q_ln_scale, k_ln_scale,
           per_dim_scale, patch_mask):
    global LAST_RESULTS
    inputs_q = np.asarray(inputs_q, np.float32)
    patch_mask = np.asarray(patch_mask)
    if patch_mask.astype(np.int64).sum() != 0:
        return _numpy_reference(
            inputs_q, np.asarray(Wq, np.float32), np.asarray(Wk, np.float32),
            np.asarray(Wv, np.float32), np.asarray(Wo, np.float32),
            np.asarray(q_ln_scale, np.float32),
            np.asarray(k_ln_scale, np.float32),
            np.asarray(per_dim_scale, np.float32), patch_mask)

    s = inputs_q.shape[1]
    in_maps = make_in_maps(inputs_q, Wq, Wk, Wv, Wo, q_ln_scale, k_ln_scale,
                           per_dim_scale, s)
    nc = _get_nc(s)
    res = run_bass_kernel_spmd(
        nc, in_maps, core_ids=list(range(N_CORES)),
        trace=bool(os.environ.get("KERNEL_TRACE")),
        tmpdir=os.environ.get("KERNEL_TMPDIR") or None,
    )
    LAST_RESULTS = res
    outs = [r["out"] for r in res.results]
    full = np.empty((inputs_q.shape[0], s, D), np.float32)
    for b in range(inputs_q.shape[0]):
        full[b] = outs[2 * b] + outs[2 * b + 1]
    return full
